# revision 40
# baseline (speedup 1.0000x reference)
"""Trainium2 Bass kernel for nn_DsbnKdLoss (segment-reduce KD loss).

Numerically exact f32 design.  The reference's final softmax/KL runs in
f32 on O(1e-3) logit averages and its output (~8e-8) is chaotic at the
~3e-8 level of the softmax inputs, so the per-class sums must be exact
f32 sums of the original logits (any 16-bit cast of the data perturbs
the bucket sums by ~0.1-1.0 and re-rolls the reference's rounding noise
into a +-20% output lottery).  The host then replays the reference's
f32 avg/softmax/KL exactly.

Sharding: data-parallel over H (8 slices of 20 rows), full (B, C)
planes per core.  Per core (raw Bass pipeline — the toolchain's walrus
accepts only one sync wait per instruction, so all synchronization is
explicit standalone wait_ge instructions):
  - SP: streams label tiles and logits planes into rotating SBUF slots.
  - DVE: one fused scalar_tensor_tensor per (side, b, i, c):
        out = (gt == i) * logits,  accum_out[p] = sum_f out[p, f]
    which is the entire segment reduction for classes 0..2.
  - ACT: per-plane totals (Copy with accum_out) -> class 3 via
    S3 = T - S0 - S1 - S2; per-label-tile moments sum(g), sum(g^2),
    sum(relu(g - 2.5)) -> exact class counts via a 4x4 solve.
  - Host: sum 128-partition partials in f64, finish in f32.
"""

import numpy as np

import concourse.bass as bass
import concourse.mybir as mybir
from concourse.bass_utils import run_bass_kernel_spmd

# Problem constants (hardcoded per contract)
B, C, H, W, Z = 4, 4, 160, 160, 96
N_CORES = 8
H_SH = H // N_CORES            # 20 rows of H per core
PLANE = H_SH * W * Z           # 307200 elements per (b, c) plane slice
P = 128
F = PLANE // P                 # 2400 free-dim elements
EPS = 1e-06
TEMPERATURE = 2.0

NG = 3                         # label-tile slots
NL = 8                         # logits-plane slots

F32 = mybir.dt.float32
BF16 = mybir.dt.bfloat16
I32 = mybir.dt.int32

AF = mybir.ActivationFunctionType
ALU = mybir.AluOpType

NB = 2 * B                     # 8 (side, b) groups
NP = NB * C                    # 32 planes
ACT_PER_GROUP = 3 + C          # 3 moments + 4 totals


def _act_after_moments(nb):
    return nb * ACT_PER_GROUP + 3


def _act_after_totals(n):
    return (n // C) * ACT_PER_GROUP + 3 + (n % C) + 1


def build_nc() -> bass.Bass:
    nc = bass.Bass()

    # Register -2.5 as a const AP (Relu moment bias) in the same
    # barrier-protected preamble style Bass uses for 0.0/1.0.
    _cm25 = nc.alloc_sbuf_tensor("const-float32-m2.5", [P, 1], F32)
    nc.gpsimd.memset(_cm25.ap(), -2.5)
    nc.const_aps.aps[(F32, -2.5)] = _cm25.ap()
    nc.all_engine_barrier()

    sl = nc.dram_tensor("src_logits", (B * C, P, F), F32, kind="ExternalInput")
    tl = nc.dram_tensor("trg_logits", (B * C, P, F), F32, kind="ExternalInput")
    sg = nc.dram_tensor("src_gt", (B, P, F), I32, kind="ExternalInput")
    tg = nc.dram_tensor("trg_gt", (B, P, F), I32, kind="ExternalInput")

    acc_out = nc.dram_tensor("acc", (P, 96), F32, kind="ExternalOutput")
    tot_out = nc.dram_tensor("tot", (P, 32), F32, kind="ExternalOutput")
    mom_out = nc.dram_tensor("mom", (P, 24), F32, kind="ExternalOutput")

    import contextlib
    with contextlib.ExitStack() as ctx:
        g_sb = [ctx.enter_context(nc.sbuf_tensor(f"g_sb{i}", [P, F], I32))
                for i in range(NG)]
        l_sb = [ctx.enter_context(nc.sbuf_tensor(f"l_sb{i}", [P, F], F32))
                for i in range(NL)]
        prod = ctx.enter_context(nc.sbuf_tensor([P, F], F32))
        scr = ctx.enter_context(nc.sbuf_tensor([P, F], BF16))
        acc_t = ctx.enter_context(nc.sbuf_tensor([P, 96], F32))
        tot_t = ctx.enter_context(nc.sbuf_tensor([P, 32], F32))
        mom_t = ctx.enter_context(nc.sbuf_tensor([P, 24], F32))
        s_g = [ctx.enter_context(nc.semaphore(name=f"s_g{i}"))
               for i in range(NG)]
        s_l = [ctx.enter_context(nc.semaphore(name=f"s_l{i}"))
               for i in range(NL)]
        s_dve = ctx.enter_context(nc.semaphore())
        s_act = ctx.enter_context(nc.semaphore())
        s_out = ctx.enter_context(nc.semaphore())
        block = ctx.enter_context(nc.Block())

        @block.sync
        def _(sp):
            for side, (lg_d, gt_d) in enumerate(((sl, sg), (tl, tg))):
                for b in range(B):
                    nb = side * B + b
                    if nb >= NG:
                        prev = nb - NG
                        # previous occupant consumed by 12 STTs + 3 moments
                        sp.wait_ge(s_dve, 12 * (prev + 1))
                        sp.wait_ge(s_act, _act_after_moments(prev))
                    sp.dma_start(g_sb[nb % NG][:], gt_d[b]).then_inc(
                        s_g[nb % NG], 16)
                    for c in range(C):
                        n = nb * C + c
                        if n >= NL:
                            prev = n - NL
                            sp.wait_ge(s_dve, 3 * (prev + 1))
                            sp.wait_ge(s_act, _act_after_totals(prev))
                        sp.dma_start(
                            l_sb[n % NL][:], lg_d[b * C + c]
                        ).then_inc(s_l[n % NL], 16)
            sp.wait_ge(s_dve, 3 * NP)
            sp.wait_ge(s_act, NB * ACT_PER_GROUP)
            sp.dma_start(acc_out[:], acc_t[:]).then_inc(s_out, 16)
            sp.dma_start(tot_out[:], tot_t[:]).then_inc(s_out, 16)
            sp.dma_start(mom_out[:], mom_t[:]).then_inc(s_out, 16)
            sp.wait_ge(s_out, 48)

        @block.vector
        def _(ve):
            d = 0
            for nb in range(NB):
                side, b = divmod(nb, B)
                ve.wait_ge(s_g[nb % NG], 16 * (nb // NG + 1))
                for c in range(C):
                    n = nb * C + c
                    ve.wait_ge(s_l[n % NL], 16 * (n // NL + 1))
                    for i in range(3):
                        col = side * 48 + b * 12 + i * 4 + c
                        if d > 0:
                            # WAW self-sync on the prod scratch (race
                            # detector requires explicit sem ordering)
                            ve.wait_ge(s_dve, d)
                        ve.scalar_tensor_tensor(
                            prod[:], g_sb[nb % NG][:], float(i), l_sb[n % NL][:],
                            ALU.is_equal, ALU.mult,
                            accum_out=acc_t[:, col : col + 1],
                        ).then_inc(s_dve, 1)
                        d += 1

        @block.scalar
        def _(ac):
            a = 0
            for nb in range(NB):
                side, b = divmod(nb, B)
                ac.wait_ge(s_g[nb % NG], 16 * (nb // NG + 1))
                mbase = side * 12 + b * 3
                for k, (func, bias) in enumerate(
                    ((AF.Copy, 0.0), (AF.Square, 0.0), (AF.Relu, -2.5))
                ):
                    if a > 0:
                        ac.wait_ge(s_act, a)
                    ac.activation(
                        out=scr[:], in_=g_sb[nb % NG][:], func=func, bias=bias,
                        accum_out=mom_t[:, mbase + k : mbase + k + 1],
                    ).then_inc(s_act, 1)
                    a += 1
                for c in range(C):
                    n = nb * C + c
                    ac.wait_ge(s_l[n % NL], 16 * (n // NL + 1))
                    tcol = side * 16 + b * 4 + c
                    if a > 0:
                        ac.wait_ge(s_act, a)
                    ac.activation(
                        out=scr[:], in_=l_sb[n % NL][:], func=AF.Copy,
                        accum_out=tot_t[:, tcol : tcol + 1],
                    ).then_inc(s_act, 1)
                    a += 1

    return nc


_NC_CACHE: list = []


def _get_nc() -> bass.Bass:
    if not _NC_CACHE:
        _NC_CACHE.append(build_nc())
    return _NC_CACHE[0]


def _shard_inputs(src_logits, trg_logits, src_gt, trg_gt):
    in_maps = []
    for j in range(N_CORES):
        h0, h1 = j * H_SH, (j + 1) * H_SH
        in_maps.append({
            "src_logits": np.ascontiguousarray(
                src_logits[:, :, h0:h1]).reshape(B * C, P, F),
            "trg_logits": np.ascontiguousarray(
                trg_logits[:, :, h0:h1]).reshape(B * C, P, F),
            "src_gt": np.ascontiguousarray(
                src_gt[:, 0, h0:h1]).reshape(B, P, F),
            "trg_gt": np.ascontiguousarray(
                trg_gt[:, 0, h0:h1]).reshape(B, P, F),
        })
    return in_maps


# Moment matrix for count recovery: rows are the per-class values of
# [1, g, g^2, relu(g-2.5)] for g = 0..3.
_MOM_MAT = np.array([
    [1.0, 1.0, 1.0, 1.0],
    [0.0, 1.0, 2.0, 3.0],
    [0.0, 1.0, 4.0, 9.0],
    [0.0, 0.0, 0.0, 0.5],
], dtype=np.float64)


def _postprocess(results) -> np.float32:
    sums = np.zeros((2, C, C), np.float64)   # [side, mask-class, channel]
    tots = np.zeros((2, C), np.float64)
    moms = np.zeros((2, 3), np.float64)      # sum g, sum g^2, sum relu(g-2.5)
    for res in results:
        acc = np.asarray(res["acc"], np.float64)
        tot = np.asarray(res["tot"], np.float64)
        mom = np.asarray(res["mom"], np.float64)
        for side in range(2):
            for b in range(B):
                for i in range(3):
                    for c in range(C):
                        col = side * 48 + b * 12 + i * 4 + c
                        sums[side, i, c] += acc[:, col].sum()
                for c in range(C):
                    tots[side, c] += tot[:, side * 16 + b * 4 + c].sum()
                for k in range(3):
                    moms[side, k] += mom[:, side * 12 + b * 3 + k].sum()

    n_total = float(B * H * W * Z)
    cnts = np.zeros((2, C), np.float64)
    for side in range(2):
        sums[side, 3] = tots[side] - sums[side, :3].sum(axis=0)
        rhs = np.array([n_total, moms[side, 0], moms[side, 1], moms[side, 2]])
        cnts[side] = np.linalg.solve(_MOM_MAT, rhs)

    # f32 finish replicating the reference's jax ops exactly
    dt = np.float32
    out = []
    for side in range(2):
        s = sums[side].astype(dt)
        cnt = cnts[side].astype(dt)
        avg = s / (cnt[:, None] + dt(EPS))
        a = avg / dt(TEMPERATURE)
        e = np.exp(a - a.max(axis=-1, keepdims=True)).astype(dt)
        p = (e / e.sum(axis=-1, keepdims=True, dtype=dt)).astype(dt)
        out.append(p)
    s_p, t_p = out
    kl = (np.sum(s_p * np.log(s_p / t_p), dtype=dt)
          + np.sum(t_p * np.log(t_p / s_p), dtype=dt))
    return np.float32((kl / dt(2.0)) / dt(C))


def kernel(src_logits, trg_logits, src_gt, trg_gt):
    nc = _get_nc()
    in_maps = _shard_inputs(src_logits, trg_logits, src_gt, trg_gt)
    res = run_bass_kernel_spmd(nc, in_maps, core_ids=list(range(N_CORES)))
    return _postprocess(res.results)


# revision 41
# speedup vs baseline: 1.0708x; 1.0708x over previous
"""Trainium2 Bass kernel for nn_DsbnKdLoss (segment-reduce KD loss).

Numerically exact f32 design.  The reference's final softmax/KL runs in
f32 on O(1e-3) logit averages and its output (~8e-8) is chaotic at the
~3e-8 level of the softmax inputs, so the per-class sums must be exact
f32 sums of the original logits (any 16-bit cast of the data perturbs
the bucket sums by ~0.1-1.0 and re-rolls the reference's rounding noise
into a +-20% output lottery).  The host then replays the reference's
f32 avg/softmax/KL exactly.

Sharding: data-parallel over H (8 slices of 20 rows), full (B, C)
planes per core.  Per core (raw Bass pipeline — the toolchain's walrus
accepts only one sync wait per instruction, so all synchronization is
explicit standalone wait_ge instructions):
  - SP: streams label tiles and logits planes into rotating SBUF slots.
  - DVE: one fused scalar_tensor_tensor per (side, b, i, c):
        out = (gt == i) * logits,  accum_out[p] = sum_f out[p, f]
    which is the entire segment reduction for classes 0..2.
  - ACT: per-plane totals (Copy with accum_out) -> class 3 via
    S3 = T - S0 - S1 - S2; per-label-tile moments sum(g), sum(g^2),
    sum(relu(g - 2.5)) -> exact class counts via a 4x4 solve.
  - Host: sum 128-partition partials in f64, finish in f32.
"""

import numpy as np

import concourse.bass as bass
import concourse.mybir as mybir
from concourse.bass_utils import run_bass_kernel_spmd

# Problem constants (hardcoded per contract)
B, C, H, W, Z = 4, 4, 160, 160, 96
N_CORES = 8
H_SH = H // N_CORES            # 20 rows of H per core
PLANE = H_SH * W * Z           # 307200 elements per (b, c) plane slice
P = 128
F = PLANE // P                 # 2400 free-dim elements
EPS = 1e-06
TEMPERATURE = 2.0

NG = 3                         # label-tile slots
NL = 8                         # logits-plane slots

F32 = mybir.dt.float32
BF16 = mybir.dt.bfloat16
I32 = mybir.dt.int32

AF = mybir.ActivationFunctionType
ALU = mybir.AluOpType

NB = 2 * B                     # 8 (side, b) groups
NP = NB * C                    # 32 planes
ACT_PER_GROUP = 3 + C          # 3 moments + 4 totals


def _act_after_moments(nb):
    return nb * ACT_PER_GROUP + 3


def _act_after_totals(n):
    return (n // C) * ACT_PER_GROUP + 3 + (n % C) + 1


def build_nc() -> bass.Bass:
    nc = bass.Bass()

    # Register -2.5 as a const AP (Relu moment bias) in the same
    # barrier-protected preamble style Bass uses for 0.0/1.0.
    _cm25 = nc.alloc_sbuf_tensor("const-float32-m2.5", [P, 1], F32)
    nc.gpsimd.memset(_cm25.ap(), -2.5)
    nc.const_aps.aps[(F32, -2.5)] = _cm25.ap()
    nc.all_engine_barrier()

    sl = nc.dram_tensor("src_logits", (B * C, P, F), F32, kind="ExternalInput")
    tl = nc.dram_tensor("trg_logits", (B * C, P, F), F32, kind="ExternalInput")
    sg = nc.dram_tensor("src_gt", (B, P, F), I32, kind="ExternalInput")
    tg = nc.dram_tensor("trg_gt", (B, P, F), I32, kind="ExternalInput")

    acc_out = nc.dram_tensor("acc", (P, 96), F32, kind="ExternalOutput")
    tot_out = nc.dram_tensor("tot", (P, 32), F32, kind="ExternalOutput")
    mom_out = nc.dram_tensor("mom", (P, 24), F32, kind="ExternalOutput")

    import contextlib
    with contextlib.ExitStack() as ctx:
        g_sb = [ctx.enter_context(nc.sbuf_tensor(f"g_sb{i}", [P, F], I32))
                for i in range(NG)]
        l_sb = [ctx.enter_context(nc.sbuf_tensor(f"l_sb{i}", [P, F], F32))
                for i in range(NL)]
        prod = [ctx.enter_context(nc.sbuf_tensor(f"prod{i}", [P, F], F32))
                for i in range(2)]
        scr = ctx.enter_context(nc.sbuf_tensor([P, F], BF16))
        acc_t = ctx.enter_context(nc.sbuf_tensor([P, 96], F32))
        tot_t = ctx.enter_context(nc.sbuf_tensor([P, 32], F32))
        mom_t = ctx.enter_context(nc.sbuf_tensor([P, 24], F32))
        s_g = [ctx.enter_context(nc.semaphore(name=f"s_g{i}"))
               for i in range(NG)]
        s_l = [ctx.enter_context(nc.semaphore(name=f"s_l{i}"))
               for i in range(NL)]
        s_dve = ctx.enter_context(nc.semaphore())
        s_act = ctx.enter_context(nc.semaphore())
        s_out = ctx.enter_context(nc.semaphore())
        block = ctx.enter_context(nc.Block())

        @block.sync
        def _(sp):
            for side, (lg_d, gt_d) in enumerate(((sl, sg), (tl, tg))):
                for b in range(B):
                    nb = side * B + b
                    if nb >= NG:
                        prev = nb - NG
                        # previous occupant consumed by 12 STTs + 3 moments
                        sp.wait_ge(s_dve, 12 * (prev + 1))
                        sp.wait_ge(s_act, _act_after_moments(prev))
                    sp.dma_start(g_sb[nb % NG][:], gt_d[b]).then_inc(
                        s_g[nb % NG], 16)
                    for c in range(C):
                        n = nb * C + c
                        if n >= NL:
                            prev = n - NL
                            sp.wait_ge(s_dve, 3 * (prev + 1))
                            sp.wait_ge(s_act, _act_after_totals(prev))
                        sp.dma_start(
                            l_sb[n % NL][:], lg_d[b * C + c]
                        ).then_inc(s_l[n % NL], 16)
            sp.wait_ge(s_dve, 3 * NP)
            sp.wait_ge(s_act, NB * ACT_PER_GROUP)
            sp.dma_start(acc_out[:], acc_t[:]).then_inc(s_out, 16)
            sp.dma_start(tot_out[:], tot_t[:]).then_inc(s_out, 16)
            sp.dma_start(mom_out[:], mom_t[:]).then_inc(s_out, 16)
            sp.wait_ge(s_out, 48)

        @block.vector
        def _(ve):
            d = 0
            for nb in range(NB):
                side, b = divmod(nb, B)
                ve.wait_ge(s_g[nb % NG], 16 * (nb // NG + 1))
                for c in range(C):
                    n = nb * C + c
                    ve.wait_ge(s_l[n % NL], 16 * (n // NL + 1))
                    for i in range(3):
                        col = side * 48 + b * 12 + i * 4 + c
                        if d > 1:
                            # WAW self-sync on the alternating prod
                            # scratches: scratch d%2 was last written by
                            # op d-2, so waiting for its completion
                            # (s_dve >= d-1) lets op d issue while op
                            # d-1 is still draining.
                            ve.wait_ge(s_dve, d - 1)
                        ve.scalar_tensor_tensor(
                            prod[d % 2][:], g_sb[nb % NG][:], float(i),
                            l_sb[n % NL][:],
                            ALU.is_equal, ALU.mult,
                            accum_out=acc_t[:, col : col + 1],
                        ).then_inc(s_dve, 1)
                        d += 1

        @block.scalar
        def _(ac):
            a = 0
            for nb in range(NB):
                side, b = divmod(nb, B)
                ac.wait_ge(s_g[nb % NG], 16 * (nb // NG + 1))
                mbase = side * 12 + b * 3
                for k, (func, bias) in enumerate(
                    ((AF.Copy, 0.0), (AF.Square, 0.0), (AF.Relu, -2.5))
                ):
                    if a > 0:
                        ac.wait_ge(s_act, a)
                    ac.activation(
                        out=scr[:], in_=g_sb[nb % NG][:], func=func, bias=bias,
                        accum_out=mom_t[:, mbase + k : mbase + k + 1],
                    ).then_inc(s_act, 1)
                    a += 1
                for c in range(C):
                    n = nb * C + c
                    ac.wait_ge(s_l[n % NL], 16 * (n // NL + 1))
                    tcol = side * 16 + b * 4 + c
                    if a > 0:
                        ac.wait_ge(s_act, a)
                    ac.activation(
                        out=scr[:], in_=l_sb[n % NL][:], func=AF.Copy,
                        accum_out=tot_t[:, tcol : tcol + 1],
                    ).then_inc(s_act, 1)
                    a += 1

    return nc


_NC_CACHE: list = []


def _get_nc() -> bass.Bass:
    if not _NC_CACHE:
        _NC_CACHE.append(build_nc())
    return _NC_CACHE[0]


def _shard_inputs(src_logits, trg_logits, src_gt, trg_gt):
    in_maps = []
    for j in range(N_CORES):
        h0, h1 = j * H_SH, (j + 1) * H_SH
        in_maps.append({
            "src_logits": np.ascontiguousarray(
                src_logits[:, :, h0:h1]).reshape(B * C, P, F),
            "trg_logits": np.ascontiguousarray(
                trg_logits[:, :, h0:h1]).reshape(B * C, P, F),
            "src_gt": np.ascontiguousarray(
                src_gt[:, 0, h0:h1]).reshape(B, P, F),
            "trg_gt": np.ascontiguousarray(
                trg_gt[:, 0, h0:h1]).reshape(B, P, F),
        })
    return in_maps


# Moment matrix for count recovery: rows are the per-class values of
# [1, g, g^2, relu(g-2.5)] for g = 0..3.
_MOM_MAT = np.array([
    [1.0, 1.0, 1.0, 1.0],
    [0.0, 1.0, 2.0, 3.0],
    [0.0, 1.0, 4.0, 9.0],
    [0.0, 0.0, 0.0, 0.5],
], dtype=np.float64)


def _postprocess(results) -> np.float32:
    sums = np.zeros((2, C, C), np.float64)   # [side, mask-class, channel]
    tots = np.zeros((2, C), np.float64)
    moms = np.zeros((2, 3), np.float64)      # sum g, sum g^2, sum relu(g-2.5)
    for res in results:
        acc = np.asarray(res["acc"], np.float64)
        tot = np.asarray(res["tot"], np.float64)
        mom = np.asarray(res["mom"], np.float64)
        for side in range(2):
            for b in range(B):
                for i in range(3):
                    for c in range(C):
                        col = side * 48 + b * 12 + i * 4 + c
                        sums[side, i, c] += acc[:, col].sum()
                for c in range(C):
                    tots[side, c] += tot[:, side * 16 + b * 4 + c].sum()
                for k in range(3):
                    moms[side, k] += mom[:, side * 12 + b * 3 + k].sum()

    n_total = float(B * H * W * Z)
    cnts = np.zeros((2, C), np.float64)
    for side in range(2):
        sums[side, 3] = tots[side] - sums[side, :3].sum(axis=0)
        rhs = np.array([n_total, moms[side, 0], moms[side, 1], moms[side, 2]])
        cnts[side] = np.linalg.solve(_MOM_MAT, rhs)

    # f32 finish replicating the reference's jax ops exactly
    dt = np.float32
    out = []
    for side in range(2):
        s = sums[side].astype(dt)
        cnt = cnts[side].astype(dt)
        avg = s / (cnt[:, None] + dt(EPS))
        a = avg / dt(TEMPERATURE)
        e = np.exp(a - a.max(axis=-1, keepdims=True)).astype(dt)
        p = (e / e.sum(axis=-1, keepdims=True, dtype=dt)).astype(dt)
        out.append(p)
    s_p, t_p = out
    kl = (np.sum(s_p * np.log(s_p / t_p), dtype=dt)
          + np.sum(t_p * np.log(t_p / s_p), dtype=dt))
    return np.float32((kl / dt(2.0)) / dt(C))


def kernel(src_logits, trg_logits, src_gt, trg_gt):
    nc = _get_nc()
    in_maps = _shard_inputs(src_logits, trg_logits, src_gt, trg_gt)
    res = run_bass_kernel_spmd(nc, in_maps, core_ids=list(range(N_CORES)))
    return _postprocess(res.results)
